# revision 27
# baseline (speedup 1.0000x reference)
"""Trainium2 Bass kernel for a binary-conv ResNet BasicBlock (training-mode BN).

Reference computation (per nn_BasicBlock_52158082843180):
    out = sign( BN2( conv3x3(sign(BN1(conv3x3(x, sign(w1)))), sign(w2)) ) + x )
with training-mode BatchNorm (batch stats over (N,H,W), biased var, eps=1e-5).

Strategy (8 NeuronCores, data-parallel over batch N=32 -> 4 images/core):
  * conv3x3 as 9 shift-matmuls on TensorE. Both input and output live in a
    58x58 zero-padded per-image layout, so every tap is a constant flat
    offset and rhs windows are contiguous.
  * x is split on the host into 3 bf16 components (hi/lo/mid). Binary
    weights (+-1) are exact in bf16/fp8, so conv1 = 3 bf16 matmul passes
    reproduces fp32 precision up to fp32-accumulation noise; conv2's inputs
    are +-1 so a single fp8 pass is exact integer arithmetic.
  * BatchNorm is sync-BN: per-core (mean, var+mean^2) from bn_stats/bn_aggr,
    one 2KB AllReduce per BN layer, then BN+sign fused into one ScalarE
    activation (Sign, scale=gamma*rstd, bias=beta-mean*scale).
  * Residual + BN2 + sign applied in-place on the conv2 output, DMA'd out.

kernel(**inputs) takes the full unsharded inputs and returns the full output.
"""

import os
import sys
import time

for _p in ("/root/.axon_site/_ro/trn_rl_repo", "/opt/trn_rl_repo"):
    if os.path.isdir(_p) and _p not in sys.path:
        sys.path.append(_p)

import numpy as np
from contextlib import ExitStack

import concourse.bass as bass
import concourse.bacc as bacc
import concourse.tile as tile
from concourse import mybir, bass_utils

# ---------------------------------------------------------------- constants
N_CORES = 8
B, C, H, W = 32, 256, 56, 56
BSH = B // N_CORES            # images per core
HP, WP = H + 2, W + 2         # padded spatial
FLAT = HP * WP                # 3364 padded pixels per image
NCH = C // 128                # channel chunks of 128 (=2)
NTAP = 3                      # 3x3 kernel
SPLITS = int(os.environ.get("KERNEL_SPLITS", "3"))  # bf16 splits of x for conv1
NQ = 4                        # quarters per image (14 output rows each)
RQ = H // NQ                  # output rows per quarter (14)
NCK = 2                       # psum chunks per quarter
RCK = RQ // NCK               # output rows per psum chunk (7)
CKW = RCK * WP                # psum chunk width incl. junk cols (406)
VCK = RCK * W                 # valid elements per chunk (392)
QROWS = RQ + 2                # padded input rows needed per quarter (16)
EPS = 1e-5

F32 = mybir.dt.float32
BF16 = mybir.dt.bfloat16
BA_DT = mybir.dt.float8e4    # binary activation storage (+-1 exact)
FP8E5 = mybir.dt.float8e5    # conv1 mid split (scaled by 2^16; weights 2^-16)
HCK = CKW // 2               # DoubleRow half-chunk output width (203)
MIDSCALE = 2.0 ** 16

_bf16_np = None
_ba_np = None


def _np_dt(dt):
    return np.dtype(mybir.dt.np(dt))


# ---------------------------------------------------------------- program
def build_nc(n_cores=N_CORES):
    nc = bacc.Bacc(
        "TRN2",
        target_bir_lowering=False,
        debug=False,
        enable_asserts=True,
        num_devices=n_cores,
    )
    # per-core DRAM I/O
    xh = nc.dram_tensor("x_hi", [BSH, NCH, 128, FLAT], BF16, kind="ExternalInput").ap()
    xl = nc.dram_tensor("x_lo", [BSH, NCH, 128, FLAT], BF16, kind="ExternalInput").ap()
    xm = (nc.dram_tensor("x_mid", [BSH, NCH, 128, FLAT], FP8E5,
                         kind="ExternalInput").ap() if SPLITS > 2 else None)
    xr = nc.dram_tensor("x_res", [BSH, NCH, 128, H * W], F32, kind="ExternalInput").ap()
    w1 = nc.dram_tensor("w1t", [NCH, 128, 9, C], BF16, kind="ExternalInput").ap()
    w1m = (nc.dram_tensor("w1m", [128, NCH, 9, C], FP8E5,
                          kind="ExternalInput").ap() if SPLITS > 2 else None)
    w2 = nc.dram_tensor("w2t", [128, NCH, 9, C], BA_DT, kind="ExternalInput").ap()
    gb = nc.dram_tensor("gb", [128, 4, NCH], F32, kind="ExternalInput").ap()
    out = nc.dram_tensor("out", [BSH, NCH, 128, H * W], F32, kind="ExternalOutput").ap()
    xsp = [xh, xl]

    with tile.TileContext(nc) as tc, ExitStack() as ctx:
        wpool = ctx.enter_context(tc.tile_pool(name="weights", bufs=1))
        big = ctx.enter_context(tc.tile_pool(name="big", bufs=1))
        xqp = ctx.enter_context(tc.tile_pool(name="xq", bufs=1))
        psum = ctx.enter_context(tc.tile_pool(name="psum", bufs=8, space="PSUM"))
        stp = ctx.enter_context(tc.tile_pool(name="stats", bufs=1))
        scrp = ctx.enter_context(tc.tile_pool(name="scr", bufs=2))
        smp = ctx.enter_context(tc.tile_pool(name="small", bufs=1))
        finp = ctx.enter_context(tc.tile_pool(name="fin", bufs=2))
        dram = ctx.enter_context(tc.tile_pool(name="dram", bufs=1, space="DRAM"))

        # ---- persistent tiles
        w1_sb = [wpool.tile([128, 9, C], BF16, tag=f"w1_{c}", name=f"w1_{c}") for c in range(NCH)]
        for c in range(NCH):
            nc.sync.dma_start(out=w1_sb[c][:], in_=w1[c])
        w2_sb = wpool.tile([128, NCH, 9, C], BA_DT, tag="w2", name="w2")
        nc.sync.dma_start(out=w2_sb[:], in_=w2[:])
        if SPLITS > 2:
            w1m_sb = wpool.tile([128, NCH, 9, C], FP8E5, tag="w1m", name="w1m")
            nc.sync.dma_start(out=w1m_sb[:], in_=w1m[:])

        # out_sb holds conv1 output (valid pixels only, f32), later reused
        # in-place for conv2 output and the final result.
        out_sb = [big.tile([128, BSH, H * W], F32, tag=f"out_{c}", name=f"out_{c}") for c in range(NCH)]
        # binary activations, padded layout, +1 guard element at each end of
        # each cin-chunk plane; merged [128, 2, *] so DoubleRow contracts both
        # chunks in one matmul
        ba_sb = big.tile([128, NCH, BSH * FLAT + 2], BA_DT, tag="ba", name="ba")
        nc.gpsimd.memset(ba_sb[:], 0.0)

        # x-quarter staging: 2 manually-rotated buffer sets; only the 2 guard
        # elements (read into junk output columns) need a one-time zero.
        QW = QROWS * WP + 2
        xq_bufs = [[[xqp.tile([128, QW], BF16,
                              tag=f"xq_{b}_{s}_{cic}", name=f"xq_{b}_{s}_{cic}")
                     for cic in range(NCH)] for s in range(2)]
                   for b in range(2)]
        for bset in xq_bufs:
            for row in bset:
                for t in row:
                    nc.vector.memset(t[:, 0:1], 0.0)
                    nc.vector.memset(t[:, QW - 1:QW], 0.0)
        if SPLITS > 2:
            xm_bufs = [xqp.tile([128, NCH, QW], FP8E5, tag=f"xm_{b}",
                                name=f"xm_{b}") for b in range(2)]
            for t in xm_bufs:
                nc.vector.memset(t[:, :, 0:1], 0.0)
                nc.vector.memset(t[:, :, QW - 1:QW], 0.0)

        gb_sb = smp.tile([128, 4, NCH], F32, tag="gb", name="gb")
        nc.sync.dma_start(out=gb_sb[:], in_=gb)
        eps_sb = smp.tile([128, 1], F32, tag="eps", name="eps")
        nc.vector.memset(eps_sb[:], EPS)

        def conv_pass(conv_idx):
            """Emit one conv's matmuls + psum->sbuf copies + bn_stats.
            conv1 reads streamed x quarters (bf16 splits); conv2 reads ba_sb.
            Returns the per-chunk stats tiles."""
            is1 = conv_idx == 1
            w_sb = w1_sb if is1 else w2_sb
            nsplit = SPLITS if is1 else 1
            nchunk = BSH * NQ * NCK
            sums = [stp.tile([128, nchunk], F32, tag=f"sum_{c}",
                             name=f"sum{conv_idx}_{c}") for c in range(NCH)]
            sqs = [stp.tile([128, nchunk], F32, tag=f"sq_{c}",
                            name=f"sq{conv_idx}_{c}") for c in range(NCH)]
            for img in range(BSH):
                for q in range(NQ):
                    if is1:
                        # stream the 16 padded input rows of this quarter
                        xq = xq_bufs[(img * NQ + q) % 2]
                        for s in range(2):
                            for cic in range(NCH):
                                nc.sync.dma_start(
                                    out=xq[s][cic][:, 1:1 + QROWS * WP],
                                    in_=xsp[s][img, cic, :,
                                               q * RQ * WP: q * RQ * WP + QROWS * WP],
                                )
                        if SPLITS > 2:
                            xmb = xm_bufs[(img * NQ + q) % 2]
                            nc.sync.dma_start(
                                out=xmb[:, :, 1:1 + QROWS * WP],
                                in_=xm[img].rearrange("j p f -> p j f")[
                                    :, :, q * RQ * WP: q * RQ * WP + QROWS * WP],
                            )
                    for coc in range(NCH):
                        pt = [psum.tile([128, CKW], F32, tag="psum", name="pt") for _ in range(NCK)]
                        cosl = slice(coc * 128, (coc + 1) * 128)
                        started = [False] * NCK
                        if is1:
                            # hi/lo bf16 passes: full-chunk matmuls per cin chunk
                            for ky in range(NTAP):
                                for kx in range(NTAP):
                                    tap = ky * NTAP + kx
                                    for cic in range(NCH):
                                        lhsT = w_sb[cic][:, tap, cosl]
                                        for s in range(2):
                                            for ck in range(NCK):
                                                # xq guard(+1) and tap col(-1) cancel
                                                off = (7 * ck + ky) * WP + kx
                                                last = (SPLITS == 2 and tap == 8
                                                        and cic == NCH - 1 and s == 1)
                                                nc.tensor.matmul(
                                                    pt[ck][:], lhsT,
                                                    xq[s][cic][:, off: off + CKW],
                                                    start=not started[ck], stop=last)
                                                started[ck] = True
                            if SPLITS > 2:
                                # mid pass: e5m2 DoubleRow (both cin chunks per MM)
                                for ky in range(NTAP):
                                    for kx in range(NTAP):
                                        tap = ky * NTAP + kx
                                        lhsTm = w1m_sb[:, :, tap, cosl]
                                        for ck in range(NCK):
                                            off = (7 * ck + ky) * WP + kx
                                            for hh in range(2):
                                                nc.tensor.matmul(
                                                    pt[ck][:, hh * HCK:(hh + 1) * HCK],
                                                    lhsTm,
                                                    xmb[:, :, off + hh * HCK:
                                                        off + hh * HCK + HCK],
                                                    perf_mode=mybir.MatmulPerfMode.DoubleRow,
                                                    start=False,
                                                    stop=(tap == 8 and hh == 1))
                        else:
                            # conv2: e4m3 DoubleRow, both cin chunks per MM
                            base = 1 + img * FLAT + q * RQ * WP
                            for ky in range(NTAP):
                                for kx in range(NTAP):
                                    tap = ky * NTAP + kx
                                    lhsT = w2_sb[:, :, tap, cosl]
                                    for ck in range(NCK):
                                        off = base + (7 * ck + ky) * WP + kx - 1
                                        for hh in range(2):
                                            nc.tensor.matmul(
                                                pt[ck][:, hh * HCK:(hh + 1) * HCK],
                                                lhsT,
                                                ba_sb[:, :, off + hh * HCK:
                                                      off + hh * HCK + HCK],
                                                perf_mode=mybir.MatmulPerfMode.DoubleRow,
                                                start=(tap == 0 and hh == 0),
                                                stop=(tap == 8 and hh == 1))
                        # evacuate psum (valid cols only); ScalarE copy also
                        # emits the chunk row-sum; DVE computes sum of squares
                        for ck in range(NCK):
                            ci = q * NCK + ck
                            sidx = (img * NQ + q) * NCK + ck
                            dst = out_sb[coc][:, img, ci * VCK:(ci + 1) * VCK]
                            dst3 = dst.rearrange("p (r w) -> p r w", w=W)
                            src3 = pt[ck][:].rearrange("p (r w) -> p r w", w=WP)[:, :, 1:1 + W]
                            nc.scalar.activation(
                                out=dst3, in_=src3,
                                func=mybir.ActivationFunctionType.Copy,
                                accum_out=sums[coc][:, sidx:sidx + 1])
                            scr = scrp.tile([128, VCK], F32, tag="scr", name="scr")
                            nc.scalar.activation(
                                out=scr[:], in_=dst,
                                func=mybir.ActivationFunctionType.Square,
                                accum_out=sqs[coc][:, sidx:sidx + 1])
            return sums, sqs

        def sync_bn(stats, tag):
            """AllReduce(sum, sumsq) -> per-channel scale/bias."""
            sums, sqs = stats
            pay = smp.tile([128, NCH, 2], F32, tag=f"pay{tag}", name=f"pay{tag}")
            for coc in range(NCH):
                nc.vector.reduce_sum(pay[:, coc, 0:1], sums[coc][:],
                                     axis=mybir.AxisListType.X)
                nc.vector.reduce_sum(pay[:, coc, 1:2], sqs[coc][:],
                                     axis=mybir.AxisListType.X)
            cin = dram.tile([128, NCH * 2], F32, tag=f"cin{tag}", name=f"cin{tag}")
            cout_ = dram.tile([128, NCH * 2], F32, tag=f"cout{tag}",
                              addr_space="Shared" if n_cores % 2 == 0 else "Local",
                              name=f"ccout{tag}")
            nc.sync.dma_start(out=cin[:], in_=pay[:].rearrange("p a b -> p (a b)"))
            nc.gpsimd.collective_compute(
                "AllReduce", mybir.AluOpType.add,
                replica_groups=[list(range(n_cores))],
                ins=[cin.opt()], outs=[cout_.opt()],
            )
            ars = smp.tile([128, NCH, 2], F32, tag=f"ars{tag}", name=f"ars{tag}")
            nc.sync.dma_start(out=ars[:].rearrange("p a b -> p (a b)"), in_=cout_[:])
            gm = smp.tile([128, NCH], F32, tag=f"gm{tag}", name=f"gm{tag}")
            gv = smp.tile([128, NCH], F32, tag=f"gv{tag}", name=f"gv{tag}")
            s_t = smp.tile([128, NCH], F32, tag=f"s{tag}", name=f"s{tag}")
            t_t = smp.tile([128, NCH], F32, tag=f"t{tag}", name=f"t{tag}")
            inv = 1.0 / (BSH * n_cores * H * W)
            nc.vector.tensor_scalar_mul(gm[:], ars[:, :, 0], inv)
            nc.vector.tensor_scalar_mul(gv[:], ars[:, :, 1], inv)
            nc.vector.tensor_mul(s_t[:], gm[:], gm[:])          # s_t = gm^2 (scratch)
            nc.vector.tensor_sub(gv[:], gv[:], s_t[:])          # gv = E[x^2]-gm^2
            nc.scalar.activation(out=gv[:], in_=gv[:],
                                 func=mybir.ActivationFunctionType.Sqrt,
                                 bias=eps_sb[:], scale=1.0)      # sqrt(var+eps)
            nc.vector.reciprocal(out=gv[:], in_=gv[:])           # rstd
            gidx, bidx = (0, 1) if tag == "1" else (2, 3)
            nc.vector.tensor_mul(s_t[:], gv[:], gb_sb[:, gidx, :])   # s = gamma*rstd
            nc.vector.tensor_mul(t_t[:], gm[:], s_t[:])
            nc.vector.tensor_sub(t_t[:], gb_sb[:, bidx, :], t_t[:])  # t = beta-gm*s
            return s_t, t_t

        # ---- conv1 -> BN1 stats -> sign -> ba_sb
        st1 = conv_pass(1)
        s1, t1 = sync_bn(st1, "1")
        for img in range(BSH):
            for coc in range(NCH):
                src = out_sb[coc][:, img, :].rearrange("p (r w) -> p r w", w=W)
                base = 1 + img * FLAT
                # strided [H,W] valid window of the padded image block
                win = ba_sb[:, coc, base + WP: base + WP + H * WP]
                win = win.rearrange("p (r w) -> p r w", w=WP)[:, :, 1:1 + W]
                nc.scalar.activation(out=win, in_=src,
                                     func=mybir.ActivationFunctionType.Sign,
                                     bias=t1[:, coc:coc + 1], scale=s1[:, coc:coc + 1])

        # ---- conv2 -> BN2 stats -> +residual -> sign -> out
        st2 = conv_pass(2)
        s2, t2 = sync_bn(st2, "2")
        for img in range(BSH):
            for coc in range(NCH):
                sl = out_sb[coc][:, img, :]
                res = finp.tile([128, H * W], F32, tag="xres", name="xres")
                nc.sync.dma_start(out=res[:], in_=xr[img, coc])
                nc.vector.tensor_scalar(
                    out=sl, in0=sl,
                    scalar1=s2[:, coc:coc + 1], scalar2=t2[:, coc:coc + 1],
                    op0=mybir.AluOpType.mult, op1=mybir.AluOpType.add,
                )
                nc.vector.tensor_add(res[:], res[:], sl)
                nc.scalar.activation(out=res[:], in_=res[:],
                                     func=mybir.ActivationFunctionType.Sign)
                nc.sync.dma_start(out=out[img, coc], in_=res[:])

    nc.compile()
    return nc


def build_floor_nc():
    """Same I/O signature, near-zero compute: calibrates dispatch overhead."""
    nc = bacc.Bacc("TRN2", target_bir_lowering=False, debug=False,
                   enable_asserts=True, num_devices=N_CORES)
    nc.dram_tensor("x_hi", [BSH, NCH, 128, FLAT], BF16, kind="ExternalInput")
    nc.dram_tensor("x_lo", [BSH, NCH, 128, FLAT], BF16, kind="ExternalInput")
    if SPLITS > 2:
        nc.dram_tensor("x_mid", [BSH, NCH, 128, FLAT], FP8E5, kind="ExternalInput")
        nc.dram_tensor("w1m", [128, NCH, 9, C], FP8E5, kind="ExternalInput")
    xr = nc.dram_tensor("x_res", [BSH, NCH, 128, H * W], F32,
                        kind="ExternalInput").ap()
    nc.dram_tensor("w1t", [NCH, 128, 9, C], BF16, kind="ExternalInput")
    nc.dram_tensor("w2t", [128, NCH, 9, C], BA_DT, kind="ExternalInput")
    nc.dram_tensor("gb", [128, 4, NCH], F32, kind="ExternalInput")
    out = nc.dram_tensor("out", [BSH, NCH, 128, H * W], F32,
                         kind="ExternalOutput").ap()
    with tile.TileContext(nc) as tc, ExitStack() as ctx:
        p = ctx.enter_context(tc.tile_pool(name="p", bufs=2))
        for img in range(BSH):
            for coc in range(NCH):
                t = p.tile([128, H * W], F32, tag="t", name="t")
                nc.sync.dma_start(out=t[:], in_=xr[img, coc])
                nc.sync.dma_start(out=out[img, coc], in_=t[:])
    nc.compile()
    return nc


# ---------------------------------------------------------------- host side
def _split3(x32):
    """f32 -> bf16 hi + bf16 lo + e5m2 mid*2^16 (residual ~2^-20 rel)."""
    import ml_dtypes
    bf = np.dtype(ml_dtypes.bfloat16)
    e5 = np.dtype(ml_dtypes.float8_e5m2)
    hi = x32.astype(bf)
    r = x32 - hi.astype(np.float32)
    lo = r.astype(bf)
    r2 = r - lo.astype(np.float32)
    mid = (r2 * MIDSCALE).astype(e5)
    return hi, lo, mid


def preprocess(x, w1, gamma1, beta1, w2, gamma2, beta2):
    """Full inputs -> list of 8 per-core in_maps."""
    x = np.asarray(x, dtype=np.float32)
    xpad = np.zeros((B, C, HP, WP), np.float32)
    xpad[:, :, 1:1 + H, 1:1 + W] = x
    hi, lo, mid = _split3(xpad)

    def wprep(w, dt, scale=1.0, merged=False):
        ws = np.sign(np.asarray(w, np.float32)) * scale  # [co, ci, ky, kx]
        wt = np.ascontiguousarray(ws.transpose(1, 2, 3, 0))  # [ci, ky, kx, co]
        wt = wt.reshape(NCH, 128, 9, C)
        if merged:  # [k, j, tap, co] for DoubleRow (contraction row k+128j)
            wt = np.ascontiguousarray(wt.transpose(1, 0, 2, 3))
        return wt.astype(_np_dt(dt))

    w1t = wprep(w1, BF16)
    w1mw = wprep(w1, FP8E5, scale=1.0 / MIDSCALE, merged=True)
    w2t = wprep(w2, BA_DT, merged=True)
    gbv = np.stack([np.asarray(a, np.float32) for a in (gamma1, beta1, gamma2, beta2)])
    gb = np.ascontiguousarray(
        gbv.reshape(4, NCH, 128).transpose(2, 0, 1))  # [128, 4, NCH]

    in_maps = []
    for c in range(N_CORES):
        sl = slice(c * BSH, (c + 1) * BSH)
        in_maps.append({
            "x_hi": np.ascontiguousarray(hi[sl]).reshape(BSH, NCH, 128, FLAT),
            "x_lo": np.ascontiguousarray(lo[sl]).reshape(BSH, NCH, 128, FLAT),
            "x_mid": np.ascontiguousarray(mid[sl]).reshape(BSH, NCH, 128, FLAT),
            "x_res": np.ascontiguousarray(x[:, :, :, :][sl]).reshape(BSH, NCH, 128, H * W),
            "w1t": w1t, "w1m": w1mw, "w2t": w2t, "gb": gb,
        })
    return in_maps


def postprocess(results):
    outs = [r["out"].reshape(BSH, C, H, W) for r in results]
    return np.concatenate(outs, axis=0).astype(np.float32)


_NC = None


def get_nc():
    global _NC
    if _NC is None:
        _NC = build_nc()
    return _NC


def kernel(**inputs):
    nc = get_nc()
    in_maps = preprocess(**inputs)
    res = bass_utils.run_bass_kernel_spmd(nc, in_maps, core_ids=list(range(N_CORES)))
    return postprocess(res.results)
